# revision 19
# baseline (speedup 1.0000x reference)
"""Trainium2 Bass kernel for nn_EnhancedSmagorinsky (B=1024, N=16384, f32).

Strategy (8 cores, pure batch-parallel, 128 batch rows per core):
- All convs become band-matrix matmuls in a position-major layout.
- Per tile of 107 output positions: PE-transpose a 127-position x window
  (plus a constant ones row used to inject biases through the matmuls),
  then chain: fused conv1 (deriv/box composed into one 9-tap kernel),
  conv2, conv3, the xdiff deriv, and the final deriv conv (computed as
  y_tile^T @ band, which lands row-major for direct store).
- ELU(z) computed exactly as max(z+1, min(exp(z), 1)) - 1 with the +1
  shift absorbed into the next layer's bias: 1 ScalarE exp + 1 fused
  DVE scalar_tensor_tensor per element.
- clip(elu(z),0,1) == clip(z,0,1) exactly, so layer 3 needs no exp.
"""
import json
import math

import numpy as np

# ---- problem constants (hardcoded; kernel.py must be self-contained) ----
B = 1024
N = 16384
NCORES = 8
BL = B // NCORES          # 128 batch rows per core
SCALE = 1.0
DX = SCALE / N
CUTOFF = 2.0 * DX
SQRT2 = float(np.sqrt(2.0))

# tile geometry
W = 127                   # x-window rows per tile (row 127 = ones)
S = 107                   # output positions per tile
PAD_L = 10
NT = math.ceil(N / S)     # 154
NEED_R = S * (NT - 1) + W - N
NPAD = PAD_L + N + NEED_R
GT = 4                    # tiles per group (free dim = GT*128 = 512)
NGROUPS = math.ceil(NT / GT)

# band-matrix column offsets inside the packed "bands" tensor
O_B1 = [co * 119 for co in range(6)]
O_XD = 6 * 119
O_B2 = [[O_XD + 111 + (ci * 3 + co) * 115 for co in range(3)] for ci in range(6)]
O_B3 = [O_XD + 111 + 18 * 115 + ci * 111 for ci in range(3)]
O_B5 = O_B3[0] + 3 * 111
O_ID = O_B5 + 256
O_ONES = O_ID + 128
NB = O_ONES + 512


def _build_constants(deriv_w, filt_w, W1, b1, W2, b2, W3, b3):
    """Pack all band matrices into one [128, NB] f32 tensor."""
    dw = np.asarray(deriv_w).reshape(5).astype(np.float64)
    bw = np.asarray(filt_w).reshape(3).astype(np.float64)
    W1 = np.asarray(W1).astype(np.float64)
    W2 = np.asarray(W2).astype(np.float64)
    W3 = np.asarray(W3).astype(np.float64)
    b1 = np.asarray(b1).astype(np.float64)
    b2 = np.asarray(b2).astype(np.float64)
    b3 = np.asarray(b3).astype(np.float64)

    F = -bw.copy()
    F[1] += 1.0
    K1 = np.zeros((6, 9))
    for co in range(6):
        K1[co] += np.convolve(W1[co, 0], dw)
        K1[co, 1:8] += np.convolve(W1[co, 1], F)
    shift1 = b1 + 1.0
    shift2 = b2 + 1.0 - W2.sum(axis=(1, 2))
    shift3 = b3[0] + 1.0 - W3.sum()

    bands = np.zeros((128, NB))
    for co in range(6):
        Bm = bands[:, O_B1[co]: O_B1[co] + 119]
        for r1 in range(119):
            for m in range(9):
                Bm[r1 + m, r1] = K1[co, m]
        Bm[127, :] = shift1[co]
    Bm = bands[:, O_XD: O_XD + 111]
    for r in range(111):
        for m in range(5):
            Bm[r + 6 + m, r] = dw[m]
    for ci in range(6):
        for co in range(3):
            Bm = bands[:, O_B2[ci][co]: O_B2[ci][co] + 115]
            for r2 in range(115):
                for m in range(5):
                    Bm[r2 + m, r2] = W2[co, ci, m]
            if ci == 5:
                Bm[119, :] = shift2[co]
    for ci in range(3):
        Bm = bands[:, O_B3[ci]: O_B3[ci] + 111]
        for r3 in range(111):
            for m in range(5):
                Bm[r3 + m, r3] = W3[0, ci, m]
        if ci == 2:
            Bm[115, :] = shift3
    Bm = bands[:, O_B5: O_B5 + 256]
    for np_ in range(107):
        for m in range(5):
            Bm[np_ + m, np_] = dw[m] * (CUTOFF ** 2) * SQRT2
    bands[:, O_ID: O_ID + 128] = np.eye(128)
    bands[0, O_ONES: O_ONES + 512] = 1.0
    return bands.astype(np.float32)


# ---- BIR fix: this walrus build allows only one embedded sync-wait per
# instruction; hoist extras onto standalone EventSemaphore instructions ----
def _split_multiwait_bir(bir_bytes: bytes) -> bytes:
    bir = json.loads(bir_bytes)
    ctr = 0
    for fn in bir.get("functions", []):
        for blk in fn.get("blocks", []):
            out = []
            for inst in blk.get("instructions", []):
                si = inst.get("sync_info")
                if si:
                    waits = si.get("on_wait") or []
                    if len(waits) > 1:
                        for w in waits[:-1]:
                            ctr += 1
                            out.append({
                                "debug": inst.get("debug", 0),
                                "engine": inst["engine"],
                                "ins": [], "outs": [],
                                "name": f"xwait-{ctr}",
                                "opcode": "EventSemaphore",
                                "sync_info": {"on_update": [], "on_wait": [w]},
                            })
                        si["on_wait"] = [waits[-1]]
                out.append(inst)
            blk["instructions"] = out
    return json.dumps(bir).encode()


_CACHED_NC = None


def _build_bass():
    global _CACHED_NC
    if _CACHED_NC is not None:
        return _CACHED_NC
    from contextlib import ExitStack

    import concourse.bass as bass
    import concourse.tile as tile
    from concourse import mybir
    from concourse.alu_op_type import AluOpType

    F32 = mybir.dt.float32
    F32R = mybir.dt.float32r  # 1 cyc/row on PE (vs 4 for fp32) when free dim >= 256
    AF = mybir.ActivationFunctionType

    nc = bass.Bass()
    xpD = nc.dram_tensor("xp", [BL, NPAD], F32R, kind="ExternalInput")
    bandsD = nc.dram_tensor("bands", [128, NB], F32R, kind="ExternalInput")
    outD = nc.dram_tensor("out", [BL, N], F32, kind="ExternalOutput")

    with ExitStack() as ctx:
        tc = ctx.enter_context(tile.TileContext(nc))
        const = ctx.enter_context(tc.tile_pool(name="const", bufs=1))
        sb = ctx.enter_context(tc.tile_pool(name="sb", bufs=2))
        ps = ctx.enter_context(tc.tile_pool(name="ps", bufs=1, space="PSUM"))

        bands = const.tile([128, NB], F32R, tag="bands")
        nc.sync.dma_start(out=bands, in_=bandsD[:])
        x_sb = const.tile([BL, NPAD], F32R, tag="x_sb")
        nch = 8
        csz = (NPAD + nch - 1) // nch
        for ci_ in range(nch):
            c0 = ci_ * csz
            c1 = min(NPAD, c0 + csz)
            nc.sync.dma_start(out=x_sb[:, c0:c1], in_=xpD[:, c0:c1])
        neg1 = const.tile([128, 1], F32, tag="neg1")
        nc.vector.memset(neg1, -1.0)
        identr = bands[:128, O_ID: O_ID + 128]

        Bxd = bands[:128, O_XD: O_XD + 111]

        # prewarm: constant ones rows live at fixed pool-slot addresses
        # (memset, not DMA: must not queue behind the big input DMAs)
        for _ in range(2):
            t_ = sb.tile([128, 512], F32R, tag="xpos")
            nc.vector.memset(t_, 1.0)
            t_ = sb.tile([120, 1024], F32R, tag="h1p2")
            nc.vector.memset(t_, 1.0)
            t_ = sb.tile([116, 512], F32R, tag="h2_2")
            nc.vector.memset(t_, 1.0)

        # Software-pipelined schedule, 4 stages deep. Per iteration `it`:
        #   front half (group it):   transposes, xdiff, conv1 + ELU
        #   fin stage  (group it-3): final deriv conv + store
        #   conv2 stage(group it-1): conv2 + ELU
        #   conv3 stage(group it-2): conv3 + y-chain
        # Every PE op is thus >=1 iteration downstream of its producers, so
        # the in-order PE queue never stalls on the elementwise chain.
        meta = {}   # g -> (t0, gt, Fc)
        h1ps = {}   # g -> [h1 pair tiles]
        h2s = {}    # g -> [h2 tiles]
        vs = {}     # g -> v tile (|xd|*xd)
        ys = {}     # g -> y tile

        def gmeta(g):
            t0 = g * GT
            gt = min(GT, NT - t0)
            return t0, gt, gt * 128

        def stage_front(g):
            t0, gt, Fc = gmeta(g)
            tp = ps.tile([128, 512], F32R, tag="tx", bufs=1)
            for j in range(gt):
                t = t0 + j
                nc.tensor.transpose(
                    tp[:127, 128 * j: 128 * (j + 1)],
                    x_sb[:, S * t: S * t + W],
                    identr,
                )
            xt = sb.tile([128, 512], F32R, tag="xpos")
            nc.vector.tensor_copy(xt[:127, :Fc], tp[:127, :Fc])
            xdm = ps.tile([128, 512], F32, tag="tx", bufs=1)
            xd = xdm[:111]
            nc.tensor.matmul(xd[:, :Fc], lhsT=Bxd, rhs=xt[:, :Fc],
                             start=True, stop=True)
            v = sb.tile([111, 512], F32, tag="v", bufs=3)   # |xd|*xd
            nc.vector.scalar_tensor_tensor(
                out=v[:, :Fc], in0=xd[:, :Fc], scalar=0.0,
                in1=xd[:, :Fc], op0=AluOpType.abs_max, op1=AluOpType.mult)
            vs[g] = v
            # conv1 (fused 9-tap) + ELU, channel-pair batched
            h1p = []
            for r in range(3):
                zp = ps.tile([128, 1024], F32, tag="z1p", bufs=2)
                for k in range(2):
                    co = 2 * r + k
                    nc.tensor.matmul(
                        zp[:119, 512 * k: 512 * k + Fc],
                        lhsT=bands[:128, O_B1[co]: O_B1[co] + 119],
                        rhs=xt[:, :Fc], start=True, stop=True)
                rows = 120 if r == 2 else 119
                hp = sb.tile([rows, 1024], F32R, tag=f"h1p{r}")
                ep = sb.tile([119, 1024], F32, tag="e1p")
                parts = [slice(0, 1024)] if Fc == 512 else [
                    slice(512 * k, 512 * k + Fc) for k in range(2)]
                for sl in parts:
                    nc.scalar.activation(ep[:, sl], zp[:119, sl],
                                         AF.Exp, bias=neg1[:119], scale=1.0)
                    nc.vector.scalar_tensor_tensor(
                        out=hp[:119, sl], in0=ep[:, sl], scalar=1.0,
                        in1=zp[:119, sl], op0=AluOpType.min,
                        op1=AluOpType.max)
                h1p.append(hp)
            h1ps[g] = h1p

        def stage_fin(g):
            t0, gt, Fc = gmeta(g)
            y = ys.pop(g)
            fin = ps.tile([128, 1024], F32, tag="z1p", bufs=2)
            njs = []
            for j in range(gt):
                nj = min(S, N - S * (t0 + j))
                if nj <= 0:
                    continue
                nc.tensor.matmul(
                    fin[:, 256 * j: 256 * (j + 1)],
                    lhsT=y[:111, 128 * j: 128 * j + 128],
                    rhs=bands[:111, O_B5: O_B5 + 256], start=True, stop=True)
                njs.append(nj)
            cols = S * len(njs)
            osb = sb.tile([128, 428], F32, tag="osb")
            src_ap = fin[:, :256 * len(njs)].rearrange(
                "p (g s) -> p g s", s=256)[:, :, :S]
            dst_ap = osb[:, :cols].rearrange("p (g s) -> p g s", s=S)
            nc.scalar.activation(dst_ap, src_ap, AF.Copy)
            if all(nj == S for nj in njs):
                nc.sync.dma_start(
                    out=outD[:, S * t0: S * t0 + cols], in_=osb[:, :cols])
            else:
                for j, nj in enumerate(njs):
                    nc.sync.dma_start(
                        out=outD[:, S * (t0 + j): S * (t0 + j) + nj],
                        in_=osb[:, S * j: S * j + nj])

        def stage_conv2(g):
            t0, gt, Fc = gmeta(g)
            h1p = h1ps.pop(g)

            def c2chain(z, co, off):
                for ci in range(6):
                    r, k = divmod(ci, 2)
                    K = 120 if ci == 5 else 119
                    nc.tensor.matmul(
                        z[:, off: off + Fc],
                        lhsT=bands[:K, O_B2[ci][co]: O_B2[ci][co] + 115],
                        rhs=h1p[r][:K, 512 * k: 512 * k + Fc],
                        start=(ci == 0), stop=(ci == 5))

            # co 0,1 batched as a pair; co 2 single (carries the ones row)
            zp = ps.tile([115, 1024], F32, tag="z2p", bufs=1)
            c2chain(zp, 0, 0)
            c2chain(zp, 1, 512)
            h2p = sb.tile([115, 1024], F32R, tag="h2p")
            ep = sb.tile([115, 1024], F32, tag="e2p")
            parts = [slice(0, 1024)] if Fc == 512 else [
                slice(512 * k, 512 * k + Fc) for k in range(2)]
            for sl in parts:
                nc.scalar.activation(ep[:, sl], zp[:, sl], AF.Exp,
                                     bias=neg1[:115], scale=1.0)
                nc.vector.scalar_tensor_tensor(
                    out=h2p[:, sl], in0=ep[:, sl], scalar=1.0,
                    in1=zp[:, sl], op0=AluOpType.min, op1=AluOpType.max)
            zs = ps.tile([115, 512], F32, tag="z2s", bufs=1)
            c2chain(zs, 2, 0)
            e = sb.tile([115, 512], F32, tag="e2", bufs=2)
            nc.scalar.activation(e[:, :Fc], zs[:, :Fc], AF.Exp,
                                 bias=neg1[:115], scale=1.0)
            h = sb.tile([116, 512], F32R, tag="h2_2")
            nc.vector.scalar_tensor_tensor(
                out=h[:115, :Fc], in0=e[:, :Fc], scalar=1.0,
                in1=zs[:, :Fc], op0=AluOpType.min, op1=AluOpType.max)
            h2s[g] = (h2p, h)

        def stage_conv3(g):
            t0, gt, Fc = gmeta(g)
            h2p, h2single = h2s.pop(g)
            v = vs.pop(g)
            z3m = ps.tile([128, 512], F32, tag="tx", bufs=1)
            z3 = z3m[:111]
            for ci in range(3):
                if ci < 2:
                    rhs = h2p[:115, 512 * ci: 512 * ci + Fc]
                    K = 115
                else:
                    rhs = h2single[:116, :Fc]
                    K = 116
                nc.tensor.matmul(
                    z3[:, :Fc], lhsT=bands[:K, O_B3[ci]: O_B3[ci] + 111],
                    rhs=rhs, start=(ci == 0), stop=(ci == 2))
            # r = relu(z3 - 1) = cs except clipped only from below;
            # min(r, 1) is folded into the two Pool ops below.
            r = sb.tile([111, 512], F32, tag="r")
            nc.scalar.activation(r[:, :Fc], z3[:, :Fc], AF.Relu,
                                 bias=neg1[:111], scale=1.0)
            t = sb.tile([111, 512], F32, tag="t")       # cs*v
            nc.gpsimd.scalar_tensor_tensor(
                out=t[:, :Fc], in0=r[:, :Fc], scalar=1.0,
                in1=v[:, :Fc], op0=AluOpType.min, op1=AluOpType.mult)
            y = sb.tile([111, 512], F32R, tag="y")      # cs*t
            nc.gpsimd.scalar_tensor_tensor(
                out=y[:, :Fc], in0=r[:, :Fc], scalar=1.0,
                in1=t[:, :Fc], op0=AluOpType.min, op1=AluOpType.mult)
            ys[g] = y

        for it in range(NGROUPS + 3):
            if it < NGROUPS:
                stage_front(it)
            if 0 <= it - 3:
                stage_fin(it - 3)
            if 0 <= it - 2 < NGROUPS:
                stage_conv3(it - 2)
            if 0 <= it - 1 < NGROUPS:
                stage_conv2(it - 1)

    orig = nc.to_json_bytes
    nc.to_json_bytes = lambda: _split_multiwait_bir(orig())
    _CACHED_NC = nc
    return nc


def kernel(**inputs) -> np.ndarray:
    from concourse.bass_utils import run_bass_kernel_spmd

    x = np.asarray(inputs["x"], dtype=np.float32)           # [1024,1,N]
    bands = _build_constants(
        inputs["deriv_w"], inputs["filt_w"], inputs["W1"], inputs["b1"],
        inputs["W2"], inputs["b2"], inputs["W3"], inputs["b3"])

    x2 = x[:, 0, :]
    xp = np.concatenate([x2[:, -PAD_L:], x2, x2[:, :NEED_R]], axis=1)
    xp = np.ascontiguousarray(xp, dtype=np.float32)

    nc = _build_bass()
    in_maps = []
    for c in range(NCORES):
        in_maps.append({
            "xp": np.ascontiguousarray(xp[c * BL:(c + 1) * BL]),
            "bands": bands,
        })
    res = run_bass_kernel_spmd(nc, in_maps, core_ids=list(range(NCORES)))
    global LAST
    LAST = res
    out = np.empty((B, 1, N), dtype=np.float32)
    for c in range(NCORES):
        out[c * BL:(c + 1) * BL, 0, :] = res.results[c]["out"]
    return out



# revision 21
# speedup vs baseline: 1.0082x; 1.0082x over previous
"""Trainium2 Bass kernel for nn_EnhancedSmagorinsky (B=1024, N=16384, f32).

Strategy (8 cores, pure batch-parallel, 128 batch rows per core):
- All convs become band-matrix matmuls in a position-major layout.
- Per tile of 107 output positions: PE-transpose a 127-position x window
  (plus a constant ones row used to inject biases through the matmuls),
  then chain: fused conv1 (deriv/box composed into one 9-tap kernel),
  conv2, conv3, the xdiff deriv, and the final deriv conv (computed as
  y_tile^T @ band, which lands row-major for direct store).
- ELU(z) computed exactly as max(z+1, min(exp(z), 1)) - 1 with the +1
  shift absorbed into the next layer's bias: 1 ScalarE exp + 1 fused
  DVE scalar_tensor_tensor per element.
- clip(elu(z),0,1) == clip(z,0,1) exactly, so layer 3 needs no exp.
"""
import json
import math

import numpy as np

# ---- problem constants (hardcoded; kernel.py must be self-contained) ----
B = 1024
N = 16384
NCORES = 8
BL = B // NCORES          # 128 batch rows per core
SCALE = 1.0
DX = SCALE / N
CUTOFF = 2.0 * DX
SQRT2 = float(np.sqrt(2.0))

# tile geometry
W = 127                   # x-window rows per tile (row 127 = ones)
S = 107                   # output positions per tile
PAD_L = 10
NT = math.ceil(N / S)     # 154
NEED_R = S * (NT - 1) + W - N
NPAD = PAD_L + N + NEED_R
GT = 4                    # tiles per group (free dim = GT*128 = 512)
NGROUPS = math.ceil(NT / GT)

# band-matrix column offsets inside the packed "bands" tensor
O_B1 = [co * 119 for co in range(6)]
O_XD = 6 * 119
O_B2 = [[O_XD + 111 + (ci * 3 + co) * 115 for co in range(3)] for ci in range(6)]
O_B3 = [O_XD + 111 + 18 * 115 + ci * 111 for ci in range(3)]
O_B5 = O_B3[0] + 3 * 111
O_ID = O_B5 + 256
O_ONES = O_ID + 128
NB = O_ONES + 512


def _build_constants(deriv_w, filt_w, W1, b1, W2, b2, W3, b3):
    """Pack all band matrices into one [128, NB] f32 tensor."""
    dw = np.asarray(deriv_w).reshape(5).astype(np.float64)
    bw = np.asarray(filt_w).reshape(3).astype(np.float64)
    W1 = np.asarray(W1).astype(np.float64)
    W2 = np.asarray(W2).astype(np.float64)
    W3 = np.asarray(W3).astype(np.float64)
    b1 = np.asarray(b1).astype(np.float64)
    b2 = np.asarray(b2).astype(np.float64)
    b3 = np.asarray(b3).astype(np.float64)

    F = -bw.copy()
    F[1] += 1.0
    K1 = np.zeros((6, 9))
    for co in range(6):
        K1[co] += np.convolve(W1[co, 0], dw)
        K1[co, 1:8] += np.convolve(W1[co, 1], F)
    shift1 = b1 + 1.0
    shift2 = b2 + 1.0 - W2.sum(axis=(1, 2))
    shift3 = b3[0] + 1.0 - W3.sum()

    bands = np.zeros((128, NB))
    for co in range(6):
        Bm = bands[:, O_B1[co]: O_B1[co] + 119]
        for r1 in range(119):
            for m in range(9):
                Bm[r1 + m, r1] = K1[co, m]
        Bm[127, :] = shift1[co]
    Bm = bands[:, O_XD: O_XD + 111]
    for r in range(111):
        for m in range(5):
            Bm[r + 6 + m, r] = dw[m]
    for ci in range(6):
        for co in range(3):
            Bm = bands[:, O_B2[ci][co]: O_B2[ci][co] + 115]
            for r2 in range(115):
                for m in range(5):
                    Bm[r2 + m, r2] = W2[co, ci, m]
            if ci == 5:
                Bm[119, :] = shift2[co]
    for ci in range(3):
        Bm = bands[:, O_B3[ci]: O_B3[ci] + 111]
        for r3 in range(111):
            for m in range(5):
                Bm[r3 + m, r3] = W3[0, ci, m]
        if ci == 2:
            Bm[115, :] = shift3
    Bm = bands[:, O_B5: O_B5 + 256]
    for np_ in range(107):
        for m in range(5):
            Bm[np_ + m, np_] = dw[m] * (CUTOFF ** 2) * SQRT2
    bands[:, O_ID: O_ID + 128] = np.eye(128)
    bands[0, O_ONES: O_ONES + 512] = 1.0
    return bands.astype(np.float32)


# ---- BIR fix: this walrus build allows only one embedded sync-wait per
# instruction; hoist extras onto standalone EventSemaphore instructions ----
def _split_multiwait_bir(bir_bytes: bytes) -> bytes:
    bir = json.loads(bir_bytes)
    ctr = 0
    for fn in bir.get("functions", []):
        for blk in fn.get("blocks", []):
            out = []
            for inst in blk.get("instructions", []):
                si = inst.get("sync_info")
                if si:
                    waits = si.get("on_wait") or []
                    if len(waits) > 1:
                        for w in waits[:-1]:
                            ctr += 1
                            out.append({
                                "debug": inst.get("debug", 0),
                                "engine": inst["engine"],
                                "ins": [], "outs": [],
                                "name": f"xwait-{ctr}",
                                "opcode": "EventSemaphore",
                                "sync_info": {"on_update": [], "on_wait": [w]},
                            })
                        si["on_wait"] = [waits[-1]]
                out.append(inst)
            blk["instructions"] = out
    return json.dumps(bir).encode()


_CACHED_NC = None


def _build_bass():
    global _CACHED_NC
    if _CACHED_NC is not None:
        return _CACHED_NC
    from contextlib import ExitStack

    import concourse.bass as bass
    import concourse.tile as tile
    from concourse import mybir
    from concourse.alu_op_type import AluOpType

    F32 = mybir.dt.float32
    F32R = mybir.dt.float32r  # 1 cyc/row on PE (vs 4 for fp32) when free dim >= 256
    AF = mybir.ActivationFunctionType

    nc = bass.Bass()
    xpD = nc.dram_tensor("xp", [BL, NPAD], F32R, kind="ExternalInput")
    bandsD = nc.dram_tensor("bands", [128, NB], F32R, kind="ExternalInput")
    outD = nc.dram_tensor("out", [BL, N], F32, kind="ExternalOutput")

    with ExitStack() as ctx:
        tc = ctx.enter_context(tile.TileContext(nc))
        const = ctx.enter_context(tc.tile_pool(name="const", bufs=1))
        sb = ctx.enter_context(tc.tile_pool(name="sb", bufs=2))
        ps = ctx.enter_context(tc.tile_pool(name="ps", bufs=1, space="PSUM"))

        bands = const.tile([128, NB], F32R, tag="bands")
        nc.sync.dma_start(out=bands, in_=bandsD[:])
        x_sb = const.tile([BL, NPAD], F32R, tag="x_sb")
        nch = 8
        csz = (NPAD + nch - 1) // nch
        for ci_ in range(nch):
            c0 = ci_ * csz
            c1 = min(NPAD, c0 + csz)
            nc.sync.dma_start(out=x_sb[:, c0:c1], in_=xpD[:, c0:c1])
        neg1 = const.tile([128, 1], F32, tag="neg1")
        nc.vector.memset(neg1, -1.0)
        identr = bands[:128, O_ID: O_ID + 128]

        Bxd = bands[:128, O_XD: O_XD + 111]

        # prewarm: constant ones rows live at fixed pool-slot addresses
        # (memset, not DMA: must not queue behind the big input DMAs)
        for _ in range(2):
            t_ = sb.tile([128, 512], F32R, tag="xpos")
            nc.vector.memset(t_, 1.0)
            t_ = sb.tile([120, 1024], F32R, tag="h1p2")
            nc.vector.memset(t_, 1.0)
            t_ = sb.tile([116, 512], F32R, tag="h2_2")
            nc.vector.memset(t_, 1.0)

        # Software-pipelined schedule, 4 stages deep. Per iteration `it`:
        #   front half (group it):   transposes, xdiff, conv1 + ELU
        #   fin stage  (group it-3): final deriv conv + store
        #   conv2 stage(group it-1): conv2 + ELU
        #   conv3 stage(group it-2): conv3 + y-chain
        # Every PE op is thus >=1 iteration downstream of its producers, so
        # the in-order PE queue never stalls on the elementwise chain.
        meta = {}   # g -> (t0, gt, Fc)
        h1ps = {}   # g -> [h1 pair tiles]
        h2s = {}    # g -> [h2 tiles]
        vs = {}     # g -> v tile (|xd|*xd)
        ys = {}     # g -> y tile

        def gmeta(g):
            t0 = g * GT
            gt = min(GT, NT - t0)
            return t0, gt, gt * 128

        def stage_front(g):
            t0, gt, Fc = gmeta(g)
            tp = ps.tile([128, 512], F32R, tag="tx", bufs=1)
            for j in range(gt):
                t = t0 + j
                nc.tensor.transpose(
                    tp[:127, 128 * j: 128 * (j + 1)],
                    x_sb[:, S * t: S * t + W],
                    identr,
                )
            xt = sb.tile([128, 512], F32R, tag="xpos")
            nc.scalar.activation(xt[:127, :Fc], tp[:127, :Fc], AF.Copy)
            xdm = ps.tile([128, 512], F32, tag="tx", bufs=1)
            xd = xdm[:111]
            nc.tensor.matmul(xd[:, :Fc], lhsT=Bxd, rhs=xt[:, :Fc],
                             start=True, stop=True)
            xde = sb.tile([111, 512], F32, tag="xde", bufs=3)
            nc.vector.tensor_copy(xde[:, :Fc], xd[:, :Fc])
            vs[g] = xde
            # conv1 (fused 9-tap) + ELU, channel-pair batched
            h1p = []
            for r in range(3):
                zp = ps.tile([128, 1024], F32, tag="z1p", bufs=2)
                for k in range(2):
                    co = 2 * r + k
                    nc.tensor.matmul(
                        zp[:119, 512 * k: 512 * k + Fc],
                        lhsT=bands[:128, O_B1[co]: O_B1[co] + 119],
                        rhs=xt[:, :Fc], start=True, stop=True)
                rows = 120 if r == 2 else 119
                hp = sb.tile([rows, 1024], F32R, tag=f"h1p{r}")
                ep = sb.tile([119, 1024], F32, tag="e1p")
                parts = [slice(0, 1024)] if Fc == 512 else [
                    slice(512 * k, 512 * k + Fc) for k in range(2)]
                for sl in parts:
                    nc.scalar.activation(ep[:, sl], zp[:119, sl],
                                         AF.Exp, bias=neg1[:119], scale=1.0)
                    nc.vector.scalar_tensor_tensor(
                        out=hp[:119, sl], in0=ep[:, sl], scalar=1.0,
                        in1=zp[:119, sl], op0=AluOpType.min,
                        op1=AluOpType.max)
                h1p.append(hp)
            h1ps[g] = h1p

        def stage_fin(g):
            t0, gt, Fc = gmeta(g)
            y = ys.pop(g)
            fin = ps.tile([128, 1024], F32, tag="z1p", bufs=2)
            njs = []
            for j in range(gt):
                nj = min(S, N - S * (t0 + j))
                if nj <= 0:
                    continue
                nc.tensor.matmul(
                    fin[:, 256 * j: 256 * (j + 1)],
                    lhsT=y[:111, 128 * j: 128 * j + 128],
                    rhs=bands[:111, O_B5: O_B5 + 256], start=True, stop=True)
                njs.append(nj)
            cols = S * len(njs)
            osb = sb.tile([128, 428], F32, tag="osb")
            src_ap = fin[:, :256 * len(njs)].rearrange(
                "p (g s) -> p g s", s=256)[:, :, :S]
            dst_ap = osb[:, :cols].rearrange("p (g s) -> p g s", s=S)
            nc.scalar.activation(dst_ap, src_ap, AF.Copy)
            if all(nj == S for nj in njs):
                nc.sync.dma_start(
                    out=outD[:, S * t0: S * t0 + cols], in_=osb[:, :cols])
            else:
                for j, nj in enumerate(njs):
                    nc.sync.dma_start(
                        out=outD[:, S * (t0 + j): S * (t0 + j) + nj],
                        in_=osb[:, S * j: S * j + nj])

        def stage_conv2(g):
            t0, gt, Fc = gmeta(g)
            h1p = h1ps.pop(g)

            def c2chain(z, co, off):
                for ci in range(6):
                    r, k = divmod(ci, 2)
                    K = 120 if ci == 5 else 119
                    nc.tensor.matmul(
                        z[:, off: off + Fc],
                        lhsT=bands[:K, O_B2[ci][co]: O_B2[ci][co] + 115],
                        rhs=h1p[r][:K, 512 * k: 512 * k + Fc],
                        start=(ci == 0), stop=(ci == 5))

            # co 0,1 batched as a pair; co 2 single (carries the ones row)
            zp = ps.tile([115, 1024], F32, tag="z2p", bufs=1)
            c2chain(zp, 0, 0)
            c2chain(zp, 1, 512)
            h2p = sb.tile([115, 1024], F32R, tag="h2p")
            ep = sb.tile([115, 1024], F32, tag="e2p")
            parts = [slice(0, 1024)] if Fc == 512 else [
                slice(512 * k, 512 * k + Fc) for k in range(2)]
            for sl in parts:
                nc.scalar.activation(ep[:, sl], zp[:, sl], AF.Exp,
                                     bias=neg1[:115], scale=1.0)
                nc.vector.scalar_tensor_tensor(
                    out=h2p[:, sl], in0=ep[:, sl], scalar=1.0,
                    in1=zp[:, sl], op0=AluOpType.min, op1=AluOpType.max)
            zs = ps.tile([115, 512], F32, tag="z2s", bufs=1)
            c2chain(zs, 2, 0)
            e = sb.tile([115, 512], F32, tag="e2", bufs=2)
            nc.scalar.activation(e[:, :Fc], zs[:, :Fc], AF.Exp,
                                 bias=neg1[:115], scale=1.0)
            h = sb.tile([116, 512], F32R, tag="h2_2")
            nc.vector.scalar_tensor_tensor(
                out=h[:115, :Fc], in0=e[:, :Fc], scalar=1.0,
                in1=zs[:, :Fc], op0=AluOpType.min, op1=AluOpType.max)
            h2s[g] = (h2p, h)

        def stage_conv3(g):
            t0, gt, Fc = gmeta(g)
            h2p, h2single = h2s.pop(g)
            v = vs.pop(g)
            z3m = ps.tile([128, 512], F32, tag="tx", bufs=1)
            z3 = z3m[:111]
            for ci in range(3):
                if ci < 2:
                    rhs = h2p[:115, 512 * ci: 512 * ci + Fc]
                    K = 115
                else:
                    rhs = h2single[:116, :Fc]
                    K = 116
                nc.tensor.matmul(
                    z3[:, :Fc], lhsT=bands[:K, O_B3[ci]: O_B3[ci] + 111],
                    rhs=rhs, start=(ci == 0), stop=(ci == 2))
            # r = relu(z3 - 1); cs = min(r, 1), folded into t1 below.
            r = sb.tile([111, 512], F32, tag="r")
            nc.scalar.activation(r[:, :Fc], z3[:, :Fc], AF.Relu,
                                 bias=neg1[:111], scale=1.0)
            t1 = sb.tile([111, 512], F32, tag="t1")     # cs*xd
            nc.gpsimd.scalar_tensor_tensor(
                out=t1[:, :Fc], in0=r[:, :Fc], scalar=1.0,
                in1=v[:, :Fc], op0=AluOpType.min, op1=AluOpType.mult)
            y = sb.tile([111, 512], F32R, tag="y")      # t1*|t1| = cs^2*|xd|*xd
            nc.gpsimd.scalar_tensor_tensor(
                out=y[:, :Fc], in0=t1[:, :Fc], scalar=0.0,
                in1=t1[:, :Fc], op0=AluOpType.abs_max, op1=AluOpType.mult)
            ys[g] = y

        for it in range(NGROUPS + 3):
            if it < NGROUPS:
                stage_front(it)
            if 0 <= it - 3:
                stage_fin(it - 3)
            if 0 <= it - 2 < NGROUPS:
                stage_conv3(it - 2)
            if 0 <= it - 1 < NGROUPS:
                stage_conv2(it - 1)

    orig = nc.to_json_bytes
    nc.to_json_bytes = lambda: _split_multiwait_bir(orig())
    _CACHED_NC = nc
    return nc


def kernel(**inputs) -> np.ndarray:
    from concourse.bass_utils import run_bass_kernel_spmd

    x = np.asarray(inputs["x"], dtype=np.float32)           # [1024,1,N]
    bands = _build_constants(
        inputs["deriv_w"], inputs["filt_w"], inputs["W1"], inputs["b1"],
        inputs["W2"], inputs["b2"], inputs["W3"], inputs["b3"])

    x2 = x[:, 0, :]
    xp = np.concatenate([x2[:, -PAD_L:], x2, x2[:, :NEED_R]], axis=1)
    xp = np.ascontiguousarray(xp, dtype=np.float32)

    nc = _build_bass()
    in_maps = []
    for c in range(NCORES):
        in_maps.append({
            "xp": np.ascontiguousarray(xp[c * BL:(c + 1) * BL]),
            "bands": bands,
        })
    res = run_bass_kernel_spmd(nc, in_maps, core_ids=list(range(NCORES)))
    global LAST
    LAST = res
    out = np.empty((B, 1, N), dtype=np.float32)
    for c in range(NCORES):
        out[c * BL:(c + 1) * BL, 0, :] = res.results[c]["out"]
    return out

